# revision 7
# baseline (speedup 1.0000x reference)
"""BTT layer kernel for Trainium2 (8 NeuronCores, data-parallel over batch).

Computes y = BTT(x; W1, W2) where
  x: (4096, 4096) fp32, W1: (64, 64, 256) fp32, W2: (64, 256, 64) fp32
  stage 1: t[b, m2, n1, r] = sum_m1 x[b, m2, m1] * W1[m2, m1, n1*4+r]
  stage 2: y[b, n1, n2]   = sum_{m2, r} t[b, m2, n1, r] * W2[n1, m2*4+r, n2]

Strategy: data-parallel over batch (512 rows per core), weights replicated.
On each core both stages run as PE matmuls; the BTT "transpose" between the
stages (batch axis m2 <-> n1) is done with SBUF->SBUF DMAs that exchange
4-partition blocks against the free dimension.

Self-contained: hardcodes all shapes; imports the Bass toolchain from
/opt/trn_rl_repo.
"""

import os
import re
import sys

import numpy as np

sys.path.insert(0, "/opt/trn_rl_repo")

import bass_rust  # noqa: E402
import concourse.bass as bass  # noqa: E402
import concourse.mybir as mybir  # noqa: E402
import concourse.tile as tile  # noqa: E402
from concourse import bass_utils  # noqa: E402

# ----------------------------------------------------------------------------
# Environment shims
# ----------------------------------------------------------------------------


def _install_walrus_single_wait_patch():
    """This container's walrus build supports only ONE sem-wait per
    instruction. TileContext attaches several (LDWEIGHTS after two DMAs, the
    kernel-tail drain). Split every multi-wait instruction: hoist all-but-one
    wait onto same-engine NoOps placed immediately before it, and emit the
    tail drain one proc at a time."""
    if getattr(tile.TileContext, "_single_wait_patched", False):
        return

    counter = [0]

    def _split_multiwait_insts(ordered):
        for insts in ordered.values():
            i = 0
            while i < len(insts):
                inst = insts[i]
                si = getattr(inst, "sync_info", None)
                if si is not None and len(si.on_wait) > 1:
                    waits = list(si.on_wait)
                    new_nops = []
                    for w in waits[:-1]:
                        counter[0] += 1
                        nop = mybir.InstNoOp(
                            name=f"waitsplit_{counter[0]}", ins=[], outs=[]
                        )
                        nop.engine = inst.engine
                        nop.sync_info = bass_rust.SyncInfo(on_wait=[w], on_update=[])
                        new_nops.append(nop)
                    inst.sync_info = bass_rust.SyncInfo(
                        on_wait=[waits[-1]], on_update=list(si.on_update)
                    )
                    insts[i:i] = new_nops
                    i += len(new_nops)
                i += 1

    orig_lower = tile.TileContext._lower_ordered_insts

    def patched_lower(self, ordered):
        _split_multiwait_insts(ordered)
        return orig_lower(self, ordered)

    def split_drain_and_barrier(self, tick_clock, wait_clock):
        gc = tick_clock.global_clock
        ticks = [int(x) for x in re.findall(r"\d+", repr(gc.copy()))]
        emitted = False
        for i, t in enumerate(ticks):
            if t > 0:
                vec = [0] * len(ticks)
                vec[i] = t
                drain_inst = self.nc.sync.drain()
                wait_clock.add_sem_waits(
                    drain_inst.ins,
                    bass_rust.ScopedClock({None: bass_rust.VectorClock(vec)}),
                )
                emitted = True
        if not emitted:
            self.nc.sync.drain()
        self.nc.all_engine_barrier()
        assert self.sems is not None
        popped = self.nc._tile_sem_poison_stack.pop()
        assert popped is self._sem_poison
        self.nc.clear_and_free_semaphores(list(self.sems.allocated().values()))
        self.nc.all_engine_barrier()

    tile.TileContext._lower_ordered_insts = patched_lower
    tile.TileContext._drain_and_barrier = split_drain_and_barrier
    tile.TileContext._single_wait_patched = True


def _install_ntff_hook():
    """Register the NTFF profiling hook (missing antenv.axon_hooks module in
    this image). Only needed when profiling; harmless otherwise."""
    import types

    if "antenv.axon_hooks" not in sys.modules:
        import antenv

        mod = types.ModuleType("antenv.axon_hooks")
        mod._hook = None
        mod.set_axon_ntff_profile_hook = lambda h: setattr(mod, "_hook", h)
        mod.get_axon_ntff_profile_hook = lambda: mod._hook
        sys.modules["antenv.axon_hooks"] = mod
        antenv.axon_hooks = mod
    m = sys.modules["antenv.axon_hooks"]
    if m._hook is None:
        try:
            from trn_agent_boot.trn_boot import _ntff_profile_via_ctypes

            m.set_axon_ntff_profile_hook(
                _ntff_profile_via_ctypes("/opt/axon/libaxon_pjrt.so")
            )
        except Exception:
            pass
    bass_utils.upload_artifacts = lambda d: d


_install_walrus_single_wait_patch()

# ----------------------------------------------------------------------------
# Problem constants / tunables
# ----------------------------------------------------------------------------

B = 4096
M1 = M2 = N1 = N2 = 64
R = 4
NCORES = 8
BP = B // NCORES  # batch rows per core (512)

# Tunables (defaults = best measured config)
B_CHUNK = int(os.environ.get("BTT_B_CHUNK", "256"))
T_DT_NAME = os.environ.get("BTT_T_DT", "float16")  # intermediate dtype
S1_DT_NAME = os.environ.get("BTT_S1_DT", "float32r")  # stage-1 operand dtype
T1_BUFS = int(os.environ.get("BTT_T1_BUFS", "1"))
T2_BUFS = int(os.environ.get("BTT_T2_BUFS", "1"))
# CoreSim's init-tracker mishandles the reorg DMA's partition-split write AP
# (writes land fine; tracking is partial). Memset t-buffers so sim validation
# can run; never enabled on perf runs.
SIM_INIT = os.environ.get("BTT_SIM_INIT", "0") == "1"


def _np_of(dt_name):
    import ml_dtypes

    return {
        "float32": np.float32,
        "float32r": np.float32,
        "float16": np.float16,
        "bfloat16": ml_dtypes.bfloat16,
    }[dt_name]


# ----------------------------------------------------------------------------
# Bass program
# ----------------------------------------------------------------------------


def build_program(b_chunk=None, t_dt_name=None, s1_dt_name=None,
                  t1_bufs=None, t2_bufs=None):
    bc = b_chunk or B_CHUNK
    tdt = getattr(mybir.dt, t_dt_name or T_DT_NAME)
    s1dt = getattr(mybir.dt, s1_dt_name or S1_DT_NAME)
    t1_bufs = t1_bufs or T1_BUFS
    t2_bufs = t2_bufs or T2_BUFS
    n_chunks = BP // bc
    f32 = mybir.dt.float32

    nc = bass.Bass(
        "TRN2",
        target_bir_lowering=False,
        debug=False,
        # The sim race detector mis-computes footprints for the reorg's
        # partition-split APs (false positives); deps verified in the
        # scheduled IR. Only affects simulation.
        detect_race_conditions=os.environ.get("BTT_NO_RACE", "0") != "1",
    )

    # Host-marshalled layouts (see _marshal_inputs):
    #   xt[p][g][b]   = x[c*BP+b, (2g + p//64)*64 + p%64]      (128, 32, BP)
    #   w1[p][g][f]   = W1[2g + p//64, p%64, f]                (128, 32, 256)
    #   w2[k][n1][j][n2] = W2[n1, (32j + k%32)*4 + k//32, n2]  (128, 64, 2, 64)
    #     (k = r*32 + m2p: T2's partition layout, chosen so the reorg DMA has
    #      naturally matching iteration orders on both sides)
    #   yt[p][q][b]   = y[c*BP+b, (2q + p//64)*64 + p%64]      (128, 32, BP)
    xt_d = nc.dram_tensor("xt", [128, 32, BP], s1dt, kind="ExternalInput")
    w1_d = nc.dram_tensor("w1", [128, 32, 256], s1dt, kind="ExternalInput")
    w2_d = nc.dram_tensor("w2", [128, 64, 2, 64], tdt, kind="ExternalInput")
    yt_d = nc.dram_tensor("yt", [128, 32, BP], f32, kind="ExternalOutput")

    with tile.TileContext(nc) as tc:
        with (
            tc.tile_pool(name="weights", bufs=1) as wpool,
            tc.tile_pool(name="xin", bufs=4) as xpool,
            tc.tile_pool(name="t1", bufs=t1_bufs) as t1pool,
            tc.tile_pool(name="t2", bufs=t2_bufs) as t2pool,
            tc.tile_pool(name="yout", bufs=4) as ypool,
            tc.tile_pool(name="ps1", bufs=6, space="PSUM") as ps1pool,
            tc.tile_pool(name="ps2", bufs=2, space="PSUM") as ps2pool,
        ):
            w1_sb = wpool.tile([128, 32, 256], s1dt, name="w1_sb")
            w2_sb = wpool.tile([128, 64, 2, 64], tdt, name="w2_sb")
            nc.sync.dma_start(w1_sb[:], w1_d[:])
            nc.sync.dma_start(w2_sb[:], w2_d[:])

            for ci in range(n_chunks):
                bsl = slice(ci * bc, (ci + 1) * bc)
                # t buffers for this chunk: T1[h][j] partition = (n1%32)*4+r,
                # free = (m2 % 32, b); T2[j] partition = (m2%32)*4+r,
                # free = (n1, b)
                t1_sb = [
                    [
                        t1pool.tile([128, 32, bc], tdt, name=f"t1_{h}_{j}",
                                    tag=f"t1_{h}_{j}")
                        for j in range(2)
                    ]
                    for h in range(2)
                ]
                t2_sb = [
                    t2pool.tile([128, 64, bc], tdt, name=f"t2_{j}", tag=f"t2_{j}")
                    for j in range(2)
                ]
                if SIM_INIT:
                    for h in range(2):
                        for j in range(2):
                            nc.gpsimd.memset(t1_sb[h][j][:], 0.0)
                    for j in range(2):
                        nc.gpsimd.memset(t2_sb[j][:], 0.0)

                def stage1(g):
                    xg = xpool.tile([128, bc], s1dt, tag="xg")
                    nc.sync.dma_start(xg[:], xt_d[:, g, bsl])
                    for p in range(2):
                        m2 = 2 * g + p
                        j, m2p = divmod(m2, 32)
                        for h in range(2):
                            ps = ps1pool.tile([128, bc], f32, tag="ps1")
                            nc.tensor.matmul(
                                ps[:],
                                w1_sb[64 * p : 64 * p + 64, g, 128 * h : 128 * h + 128],
                                xg[64 * p : 64 * p + 64, :],
                                start=True,
                                stop=True,
                            )
                            nc.any.tensor_copy(t1_sb[h][j][:, m2p, :], ps[:])

                def reorg(j):
                    # T1[h][j] partitions = (n1%32)*4 + r, free = (m2p, b).
                    # T2[j] partitions = r*32 + m2p, free = (n1, b).
                    # src iterates (r, m2p, b), dst iterates (q, b) with
                    # q = r*32+m2p -- identical element order, both APs have
                    # the partition dim first with step 1 (HW requirement).
                    for n1 in range(N1):
                        h, n1p = divmod(n1, 32)
                        src = t1_sb[h][j][n1p * R : (n1p + 1) * R, :, :]
                        dst = t2_sb[j][:, n1, :]
                        nc.sync.dma_start(dst, src)

                for g in range(16):
                    stage1(g)
                reorg(0)
                for g in range(16, 32):
                    stage1(g)
                reorg(1)

                for q in range(32):
                    ps = ps2pool.tile([128, bc], f32, tag="ps2")
                    for p in range(2):
                        n1 = 2 * q + p
                        for j in range(2):
                            nc.tensor.matmul(
                                ps[64 * p : 64 * p + 64, :],
                                w2_sb[:, n1, j, :],
                                t2_sb[j][:, n1, :],
                                start=(j == 0),
                                stop=(j == 1),
                                tile_position=(0, 64 * p),
                            )
                    yq = ypool.tile([128, bc], f32, tag="yq")
                    nc.any.tensor_copy(yq[:], ps[:])
                    nc.sync.dma_start(yt_d[:, q, bsl], yq[:])

    return nc


# ----------------------------------------------------------------------------
# Host marshalling
# ----------------------------------------------------------------------------


def _marshal_inputs(x, W1, W2, t_np, s1_np):
    """Build per-core input maps. Layout docs in build_program."""
    # x: (B, 4096) -> per core xt (128, 32, BP)
    # row index (m2*64+m1) -> partition p = m1 + 64*(m2%2), g = m2//2
    xr = x.reshape(B, 32, 2, 64)  # (b, g, par, m1)
    xt_all = np.ascontiguousarray(
        xr.transpose(2, 3, 1, 0).reshape(128, 32, B).astype(s1_np, copy=False)
    )  # [par*64+m1][g][b]
    # W1 (64, 64, 256): w1[p][g][f] = W1[2g + p//64, p%64, f]
    w1 = np.ascontiguousarray(
        W1.reshape(32, 2, 64, 256).transpose(1, 2, 0, 3).reshape(128, 32, 256)
    ).astype(s1_np, copy=False)
    # W2 (64, 256, 64): w2[k][n1][j][n2] = W2[n1, (32j + k%32)*4 + k//32, n2]
    # (k = r*32 + m2p; W2's m2r index = (32j + m2p)*4 + r)
    w2r = W2.reshape(64, 2, 32, 4, 64)  # [n1][j][m2p][r][n2]
    w2 = np.ascontiguousarray(
        w2r.transpose(3, 2, 0, 1, 4).reshape(128, 64, 2, 64)
    ).astype(t_np, copy=False)

    in_maps = []
    for c in range(NCORES):
        in_maps.append(
            {
                "xt": np.ascontiguousarray(xt_all[:, :, c * BP : (c + 1) * BP]),
                "w1": w1,
                "w2": w2,
            }
        )
    return in_maps


def _unmarshal_output(results):
    """yt (128, 32, BP) per core -> y (B, 4096)."""
    y = np.empty((B, N1 * N2), np.float32)
    for c, res in enumerate(results):
        yt = res["yt"]  # [p][q][b], p = n2 + 64*(n1%2), q = n1//2
        # y[c*BP+b, (2q+pp)*64+n2] = yt[pp*64+n2, q, b]
        yc = yt.reshape(2, 64, 32, BP).transpose(3, 2, 0, 1).reshape(BP, 4096)
        y[c * BP : (c + 1) * BP] = yc
    return y


# ----------------------------------------------------------------------------
# Public entry point
# ----------------------------------------------------------------------------

_PROGRAM_CACHE = {}


def kernel(x, W1, W2, _trace=False, _config=None):
    cfg = _config or {}
    key = tuple(sorted(cfg.items())) if cfg else None
    if key not in _PROGRAM_CACHE:
        _PROGRAM_CACHE[key] = build_program(**cfg)
    nc = _PROGRAM_CACHE[key]

    t_np = _np_of(cfg.get("t_dt_name", T_DT_NAME))
    s1_np = _np_of(cfg.get("s1_dt_name", S1_DT_NAME))
    in_maps = _marshal_inputs(
        np.asarray(x, np.float32), np.asarray(W1, np.float32),
        np.asarray(W2, np.float32), t_np, s1_np
    )
    if _trace:
        _install_ntff_hook()
        os.environ["BASS_PERFETTO_PROFILE_ALL_CORES"] = "1"
    res = bass_utils.run_bass_kernel_spmd(
        nc, in_maps, core_ids=list(range(NCORES)), trace=_trace
    )
    y = _unmarshal_output(res.results)
    if _trace:
        return y, res
    return y


# revision 16
# speedup vs baseline: 1.7192x; 1.7192x over previous
"""BTT layer kernel for Trainium2 (8 NeuronCores, data-parallel over batch).

Computes y = BTT(x; W1, W2) where
  x: (4096, 4096) fp32, W1: (64, 64, 256) fp32, W2: (64, 256, 64) fp32
  stage 1: t[b, m2, n1, r] = sum_m1 x[b, m2, m1] * W1[m2, m1, n1*4+r]
  stage 2: y[b, n1, n2]   = sum_{m2, r} t[b, m2, n1, r] * W2[n1, m2*4+r, n2]

Strategy: data-parallel over batch (512 rows per core), weights replicated.
On each core both stages run as PE matmuls; the BTT "transpose" between the
stages (batch axis m2 <-> n1) is done with SBUF->SBUF DMAs that exchange
4-partition blocks against the free dimension.

Self-contained: hardcodes all shapes; imports the Bass toolchain from
/opt/trn_rl_repo.
"""

import os
import re
import sys

import numpy as np

sys.path.insert(0, "/opt/trn_rl_repo")

import bass_rust  # noqa: E402
import concourse.bass as bass  # noqa: E402
import concourse.mybir as mybir  # noqa: E402
import concourse.tile as tile  # noqa: E402
from concourse import bass_utils  # noqa: E402

# ----------------------------------------------------------------------------
# Environment shims
# ----------------------------------------------------------------------------


def _install_walrus_single_wait_patch():
    """This container's walrus build supports only ONE sem-wait per
    instruction. TileContext attaches several (LDWEIGHTS after two DMAs, the
    kernel-tail drain). Split every multi-wait instruction: hoist all-but-one
    wait onto same-engine NoOps placed immediately before it, and emit the
    tail drain one proc at a time."""
    if getattr(tile.TileContext, "_single_wait_patched", False):
        return

    counter = [0]

    def _split_multiwait_insts(ordered):
        for insts in ordered.values():
            i = 0
            while i < len(insts):
                inst = insts[i]
                si = getattr(inst, "sync_info", None)
                if si is not None and len(si.on_wait) > 1:
                    waits = list(si.on_wait)
                    new_nops = []
                    for w in waits[:-1]:
                        counter[0] += 1
                        nop = mybir.InstNoOp(
                            name=f"waitsplit_{counter[0]}", ins=[], outs=[]
                        )
                        nop.engine = inst.engine
                        nop.sync_info = bass_rust.SyncInfo(on_wait=[w], on_update=[])
                        new_nops.append(nop)
                    inst.sync_info = bass_rust.SyncInfo(
                        on_wait=[waits[-1]], on_update=list(si.on_update)
                    )
                    insts[i:i] = new_nops
                    i += len(new_nops)
                i += 1

    orig_lower = tile.TileContext._lower_ordered_insts

    def patched_lower(self, ordered):
        _split_multiwait_insts(ordered)
        return orig_lower(self, ordered)

    def split_drain_and_barrier(self, tick_clock, wait_clock):
        gc = tick_clock.global_clock
        ticks = [int(x) for x in re.findall(r"\d+", repr(gc.copy()))]
        emitted = False
        for i, t in enumerate(ticks):
            if t > 0:
                vec = [0] * len(ticks)
                vec[i] = t
                drain_inst = self.nc.sync.drain()
                wait_clock.add_sem_waits(
                    drain_inst.ins,
                    bass_rust.ScopedClock({None: bass_rust.VectorClock(vec)}),
                )
                emitted = True
        if not emitted:
            self.nc.sync.drain()
        self.nc.all_engine_barrier()
        assert self.sems is not None
        popped = self.nc._tile_sem_poison_stack.pop()
        assert popped is self._sem_poison
        self.nc.clear_and_free_semaphores(list(self.sems.allocated().values()))
        self.nc.all_engine_barrier()

    tile.TileContext._lower_ordered_insts = patched_lower
    tile.TileContext._drain_and_barrier = split_drain_and_barrier
    tile.TileContext._single_wait_patched = True


def _install_ntff_hook():
    """Register the NTFF profiling hook (missing antenv.axon_hooks module in
    this image). Only needed when profiling; harmless otherwise."""
    import types

    if "antenv.axon_hooks" not in sys.modules:
        import antenv

        mod = types.ModuleType("antenv.axon_hooks")
        mod._hook = None
        mod.set_axon_ntff_profile_hook = lambda h: setattr(mod, "_hook", h)
        mod.get_axon_ntff_profile_hook = lambda: mod._hook
        sys.modules["antenv.axon_hooks"] = mod
        antenv.axon_hooks = mod
    m = sys.modules["antenv.axon_hooks"]
    if m._hook is None:
        try:
            from trn_agent_boot.trn_boot import _ntff_profile_via_ctypes

            m.set_axon_ntff_profile_hook(
                _ntff_profile_via_ctypes("/opt/axon/libaxon_pjrt.so")
            )
        except Exception:
            pass
    bass_utils.upload_artifacts = lambda d: d


_install_walrus_single_wait_patch()

# ----------------------------------------------------------------------------
# Problem constants / tunables
# ----------------------------------------------------------------------------

B = 4096
M1 = M2 = N1 = N2 = 64
R = 4
NCORES = 8
BP = B // NCORES  # batch rows per core (512)

# Tunables (defaults = best measured config)
B_CHUNK = int(os.environ.get("BTT_B_CHUNK", "256"))
T_DT_NAME = os.environ.get("BTT_T_DT", "float16")  # intermediate dtype
S1_DT_NAME = os.environ.get("BTT_S1_DT", "float32r")  # stage-1 operand dtype
T1_BUFS = int(os.environ.get("BTT_T1_BUFS", "1"))
T2_BUFS = int(os.environ.get("BTT_T2_BUFS", "1"))
# CoreSim's init-tracker mishandles the reorg DMA's partition-split write AP
# (writes land fine; tracking is partial). Memset t-buffers so sim validation
# can run; never enabled on perf runs.
SIM_INIT = os.environ.get("BTT_SIM_INIT", "0") == "1"


def _np_of(dt_name):
    import ml_dtypes

    return {
        "float32": np.float32,
        "float32r": np.float32,
        "float16": np.float16,
        "bfloat16": ml_dtypes.bfloat16,
    }[dt_name]


# ----------------------------------------------------------------------------
# Bass program
# ----------------------------------------------------------------------------


def build_program(b_chunk=None, t_dt_name=None, s1_dt_name=None,
                  t1_bufs=None, t2_bufs=None):
    bc = b_chunk or B_CHUNK
    tdt = getattr(mybir.dt, t_dt_name or T_DT_NAME)
    s1dt = getattr(mybir.dt, s1_dt_name or S1_DT_NAME)
    t1_bufs = t1_bufs or T1_BUFS
    t2_bufs = t2_bufs or T2_BUFS
    n_chunks = BP // bc
    f32 = mybir.dt.float32

    nc = bass.Bass(
        "TRN2",
        target_bir_lowering=False,
        debug=False,
        # The sim race detector mis-computes footprints for the reorg's
        # partition-split APs (false positives); deps verified in the
        # scheduled IR. Only affects simulation.
        detect_race_conditions=os.environ.get("BTT_NO_RACE", "0") != "1",
    )

    # Host-marshalled layouts (see _marshal_inputs). Chunk-major so every
    # DRAM<->SBUF DMA has long contiguous runs per partition (descriptor
    # count on the issuing engine is the scarce resource):
    #   xt[p][ci][g][b'] = x[c*BP + ci*bc + b', (2g + p//64)*64 + p%64]
    #   w1[p][g][f]      = W1[2g + p//64, p%64, f]             (128, 32, 256)
    #   w2[k][n1][j][n2] = W2[n1, (32j + k%32)*4 + k//32, n2]  (128, 64, 2, 64)
    #     (k = r*32 + m2p: T2's partition layout, chosen so the reorg DMA has
    #      naturally matching iteration orders on both sides)
    #   yt[p][ci][q][b'] = y[c*BP + ci*bc + b', (2q + p//64)*64 + p%64]
    xt_d = nc.dram_tensor("xt", [128, n_chunks, 32, bc], s1dt, kind="ExternalInput")
    w1_d = nc.dram_tensor("w1", [128, 32, 256], s1dt, kind="ExternalInput")
    w2_d = nc.dram_tensor("w2", [128, 64, 2, 64], tdt, kind="ExternalInput")
    yt_d = nc.dram_tensor("yt", [128, n_chunks, 32, bc], f32, kind="ExternalOutput")

    with tile.TileContext(nc) as tc:
        with (
            tc.tile_pool(name="weights", bufs=1) as wpool,
            tc.tile_pool(name="xin", bufs=2) as xpool,
            tc.tile_pool(name="t1", bufs=t1_bufs) as t1pool,
            tc.tile_pool(name="t2", bufs=t2_bufs) as t2pool,
            tc.tile_pool(name="yout", bufs=2) as ypool,
            tc.tile_pool(name="ps1", bufs=6, space="PSUM") as ps1pool,
            tc.tile_pool(name="ps2", bufs=2, space="PSUM") as ps2pool,
        ):
            w1_sb = wpool.tile([128, 32, 256], s1dt, name="w1_sb")
            w2_sb = wpool.tile([128, 64, 2, 64], tdt, name="w2_sb")
            nc.sync.dma_start(w1_sb[:], w1_d[:])
            nc.sync.dma_start(w2_sb[:], w2_d[:])

            for ci in range(n_chunks):
                # t buffers for this chunk: T1[h][j] partition = (n1%32)*4+r,
                # free = (m2 % 32, b); T2[j] partition = r*32 + (m2%32),
                # free = (n1, b)
                t1_sb = [
                    [
                        t1pool.tile([128, 32, bc], tdt, name=f"t1_{h}_{j}",
                                    tag=f"t1_{h}_{j}")
                        for j in range(2)
                    ]
                    for h in range(2)
                ]
                t2_sb = [
                    t2pool.tile([128, 64, bc], tdt, name=f"t2_{j}", tag=f"t2_{j}")
                    for j in range(2)
                ]
                if SIM_INIT:
                    for h in range(2):
                        for j in range(2):
                            nc.gpsimd.memset(t1_sb[h][j][:], 0.0)
                    for j in range(2):
                        nc.gpsimd.memset(t2_sb[j][:], 0.0)

                def stage1(g, xg):
                    for p in range(2):
                        m2 = 2 * g + p
                        j, m2p = divmod(m2, 32)
                        for h in range(2):
                            ps = ps1pool.tile([128, bc], f32, tag="ps1")
                            nc.tensor.matmul(
                                ps[:],
                                w1_sb[64 * p : 64 * p + 64, g, 128 * h : 128 * h + 128],
                                xg[64 * p : 64 * p + 64, g % XG, :],
                                start=True,
                                stop=True,
                            )
                            nc.any.tensor_copy(t1_sb[h][j][:, m2p, :], ps[:])

                def reorg(j):
                    # T1[h][j] partitions = (n1%32)*4 + r, free = (m2p, b).
                    # T2[j] partitions = r*32 + m2p, free = (n1, b).
                    # src iterates (r, m2p, b), dst iterates (q, b) with
                    # q = r*32+m2p -- identical element order, both APs have
                    # the partition dim first with step 1 (HW requirement).
                    # Spread descriptor-writing across all 3 DMA-capable
                    # engines (SP / ACT / POOL).
                    for n1 in range(N1):
                        h, n1p = divmod(n1, 32)
                        src = t1_sb[h][j][n1p * R : (n1p + 1) * R, :, :]
                        dst = t2_sb[j][:, n1, :]
                        eng = (nc.sync, nc.scalar, nc.gpsimd)[n1 % 3]
                        eng.dma_start(dst, src)

                XG = 8  # x-load group size (g per DMA)
                for g in range(32):
                    if g % XG == 0:
                        xg = xpool.tile([128, XG, bc], s1dt, tag="xg")
                        nc.sync.dma_start(
                            xg[:], xt_d[:, ci, g : g + XG, :]
                        )
                    stage1(g, xg)
                    if g == 15:
                        reorg(0)
                reorg(1)

                YG = 4  # y-store group size (q per DMA)
                for q in range(32):
                    if q % YG == 0:
                        ysb = ypool.tile([128, YG, bc], f32, tag="ysb")
                    ps = ps2pool.tile([128, bc], f32, tag="ps2")
                    for p in range(2):
                        n1 = 2 * q + p
                        for j in range(2):
                            nc.tensor.matmul(
                                ps[64 * p : 64 * p + 64, :],
                                w2_sb[:, n1, j, :],
                                t2_sb[j][:, n1, :],
                                start=(j == 0),
                                stop=(j == 1),
                                tile_position=(0, 64 * p),
                            )
                    nc.any.tensor_copy(ysb[:, q % YG, :], ps[:])
                    if q % YG == YG - 1:
                        nc.sync.dma_start(
                            yt_d[:, ci, q - YG + 1 : q + 1, :], ysb[:]
                        )

    return nc


# ----------------------------------------------------------------------------
# Host marshalling
# ----------------------------------------------------------------------------


def _marshal_inputs(x, W1, W2, t_np, s1_np, bc):
    """Build per-core input maps. Layout docs in build_program."""
    n_chunks = BP // bc
    # x: (B, 4096) -> per core xt (128, n_chunks, 32, bc)
    # col index (m2*64+m1) -> partition p = m1 + 64*(m2%2), g = m2//2
    xr = x.reshape(B, 32, 2, 64)  # (b, g, par, m1)
    xt_all = np.ascontiguousarray(
        xr.transpose(2, 3, 1, 0).reshape(128, 32, B).astype(s1_np, copy=False)
    )  # [par*64+m1][g][b]
    # W1 (64, 64, 256): w1[p][g][f] = W1[2g + p//64, p%64, f]
    w1 = np.ascontiguousarray(
        W1.reshape(32, 2, 64, 256).transpose(1, 2, 0, 3).reshape(128, 32, 256)
    ).astype(s1_np, copy=False)
    # W2 (64, 256, 64): w2[k][n1][j][n2] = W2[n1, (32j + k%32)*4 + k//32, n2]
    # (k = r*32 + m2p; W2's m2r index = (32j + m2p)*4 + r)
    w2r = W2.reshape(64, 2, 32, 4, 64)  # [n1][j][m2p][r][n2]
    w2 = np.ascontiguousarray(
        w2r.transpose(3, 2, 0, 1, 4).reshape(128, 64, 2, 64)
    ).astype(t_np, copy=False)

    in_maps = []
    for c in range(NCORES):
        xc = xt_all[:, :, c * BP : (c + 1) * BP]  # (128, 32, BP)
        # -> (128, n_chunks, 32, bc): [p][g][ci*bc+b'] -> [p][ci][g][b']
        xc = np.ascontiguousarray(
            xc.reshape(128, 32, n_chunks, bc).transpose(0, 2, 1, 3)
        )
        in_maps.append({"xt": xc, "w1": w1, "w2": w2})
    return in_maps


def _unmarshal_output(results, bc):
    """yt (128, n_chunks, 32, bc) per core -> y (B, 4096)."""
    n_chunks = BP // bc
    y = np.empty((B, N1 * N2), np.float32)
    for c, res in enumerate(results):
        yt = res["yt"]  # [p][ci][q][b'], p = n2 + 64*(n1%2), q = n1//2
        # y[c*BP + ci*bc + b', (2q+pp)*64+n2] = yt[pp*64+n2, ci, q, b']
        yc = (
            yt.reshape(2, 64, n_chunks, 32, bc)
            .transpose(2, 4, 3, 0, 1)  # (ci, b', q, pp, n2)
            .reshape(BP, 4096)
        )
        y[c * BP : (c + 1) * BP] = yc
    return y


# ----------------------------------------------------------------------------
# Public entry point
# ----------------------------------------------------------------------------

_PROGRAM_CACHE = {}


def kernel(x, W1, W2, _trace=False, _config=None):
    cfg = _config or {}
    key = tuple(sorted(cfg.items())) if cfg else None
    if key not in _PROGRAM_CACHE:
        _PROGRAM_CACHE[key] = build_program(**cfg)
    nc = _PROGRAM_CACHE[key]

    t_np = _np_of(cfg.get("t_dt_name", T_DT_NAME))
    s1_np = _np_of(cfg.get("s1_dt_name", S1_DT_NAME))
    bc = cfg.get("b_chunk", B_CHUNK)
    in_maps = _marshal_inputs(
        np.asarray(x, np.float32), np.asarray(W1, np.float32),
        np.asarray(W2, np.float32), t_np, s1_np, bc
    )
    if _trace:
        _install_ntff_hook()
        os.environ["BASS_PERFETTO_PROFILE_ALL_CORES"] = "1"
    res = bass_utils.run_bass_kernel_spmd(
        nc, in_maps, core_ids=list(range(NCORES)), trace=_trace
    )
    y = _unmarshal_output(res.results, bc)
    if _trace:
        return y, res
    return y
